# revision 38
# baseline (speedup 1.0000x reference)
"""MultiHeadCrossAttention on 8 TRN2 NeuronCores.

Sharding: tensor-parallel over heads (16 heads -> 2 per core); each core
writes a full-size partial of y.T which the host sums (replaces the
all-reduce).

Design v2 (from the 188.5us f32->bf16/fp8 baseline).  The baseline was
PE-dispatch-bound: 2432 Matmult+Ldweights pairs (~74ns sequencer cost each,
PE.SEQ ~183us) on top of PE.ENGINE ~151us.  Changes:

  * V projection runs in NORMAL orientation (stationary = Wv, moving = x2
    chunk, 512-col passes) exactly like K: 192 DR matmuls instead of 768
    tiny swapped-role ones.  The [kv, d] layout attnV needs is recovered
    with 64 transposing DMAs (DmaTransposeAnt, ~112ns each on the DMA
    track, zero PE cost) into a staging tile + a cheap 2x-mode DVE copy
    into the 65-stride [V|1] layout.
  * att tiles are DMA-transposed too (b < 3): -24 PE transpose pairs and
    -24 DVE copies; the last batch keeps the low-latency PE-transpose path
    so the tail stays short.
  * y stores merge per (b, o): two [128,512] DVE copies into one
    [128,1024] SBUF tile, one DMA (24 instead of 64 y DMAs on the SP
    queue); b=3 keeps per-(g,o) stores on the ACT queue (idle at the tail).
  * Startup: wp1 loads in two pieces (wk plane first) and x2(0,0) in two
    halves so the first K-proj matmul issues at ~3.5us instead of ~7.4us.

  (Kept from v1: bf16 activations, fp8 DoubleRow hi/lo projections with
  x32-prescaled weights, scores via 64-deep bf16 matmuls, attnV with the
  softmax-denominator ones column, per-partition normalize, exp on ACT as
  the pacing stream with windows of 16 scores+exp steps interleaved with
  the previous window's attnV and a filler queue of projection work.)
"""
import numpy as np
import ml_dtypes
from collections import deque
from contextlib import ExitStack

import concourse.bass as bass
import concourse.mybir as mybir
import concourse.tile as tile
from concourse import bacc
from concourse.bass_utils import run_bass_kernel_spmd

N_CORES = 8
B, SQ, SKV, E, DH = 4, 1024, 2048, 1024, 64
Q_ROWS = B * SQ      # 4096
KV_ROWS = B * SKV    # 8192
EC = E // 128        # 8 contraction chunks
QC = Q_ROWS // 512   # 8 q column chunks
KVC_B = SKV // 128   # 16 kv blocks per batch
F32 = mybir.dt.float32
BF16 = mybir.dt.bfloat16
FP8 = mybir.dt.float8e4
DR = mybir.MatmulPerfMode.DoubleRow
Exp = mybir.ActivationFunctionType.Exp
SHIFT = 0.0
import os
ATT_DMA_T = os.environ.get("ATT_DMA_T", "0") == "1"   # att_T via DMA transpose
V_DMA_T = os.environ.get("V_DMA_T", "0") == "1"       # V via DMA transpose

_CACHE = {}


def _build():
    nc = bacc.Bacc("TRN2", target_bir_lowering=False, debug=False,
                   num_devices=N_CORES)
    # x slabs as fp8 hi/lo pairs (same bytes as bf16, but projections can run
    # DoubleRow: 2 contraction chunks per pass at 0.5 cyc/row)
    x1t = nc.dram_tensor("x1t", [QC, 128, 2, EC, 512], FP8,
                         kind="ExternalInput").ap()
    x2t = nc.dram_tensor("x2t", [KV_ROWS // 512, 128, 2, EC, 512], FP8,
                         kind="ExternalInput").ap()
    # packed weights:
    # wp1 = [Wk hi|lo fp8 | Wq hi|lo fp8 | bk | bq | bv]  (f32 bias bytes in
    #        bf16 slots), loaded as two DMAs (wk plane first)
    # wp2 = [Wv hi|lo fp8 | Wo.T bf16 | identity bf16]
    wp1 = nc.dram_tensor("wp1", [128, E + E + 6], BF16,
                         kind="ExternalInput").ap()
    wp2 = nc.dram_tensor("wp2", [128, E + E + 128], BF16,
                         kind="ExternalInput").ap()
    yt = nc.dram_tensor("yt", [E, Q_ROWS], BF16, kind="ExternalOutput").ap()
    yt_r = yt.rearrange("(oc p) q -> p oc q", p=128)
    DBG = os.environ.get("KDBG", "0") == "1"
    if DBG:
        dbg_qt = nc.dram_tensor("dbg_qt", [128, QC, 512], BF16,
                                kind="ExternalOutput").ap()
        dbg_kt = nc.dram_tensor("dbg_kt", [B, 128, SKV], BF16,
                                kind="ExternalOutput").ap()
        dbg_v = nc.dram_tensor("dbg_v", [B, 128, KVC_B, 130], BF16,
                               kind="ExternalOutput").ap()
        dbg_pt = nc.dram_tensor("dbg_pt", [128, KVC_B, SQ], BF16,
                                kind="ExternalOutput").ap()
        dbg_at = nc.dram_tensor("dbg_at", [B, 128, 8, 128], BF16,
                                kind="ExternalOutput").ap()
        dbg_aT = nc.dram_tensor("dbg_aT", [B, 128, SQ], BF16,
                                kind="ExternalOutput").ap()

    with tile.TileContext(nc) as tc, ExitStack() as ctx:
        const = ctx.enter_context(tc.tile_pool(name="const", bufs=1))
        persist = ctx.enter_context(tc.tile_pool(name="persist", bufs=1))
        ptp = ctx.enter_context(tc.tile_pool(name="ptp", bufs=2))
        xload = ctx.enter_context(tc.tile_pool(name="xload", bufs=6))
        work = ctx.enter_context(tc.tile_pool(name="work", bufs=3))
        ps_pj = ctx.enter_context(tc.tile_pool(name="ps_pj", bufs=2, space="PSUM"))
        ps_s = ctx.enter_context(tc.tile_pool(name="ps_s", bufs=2, space="PSUM"))
        ps_o = ctx.enter_context(tc.tile_pool(name="ps_o", bufs=2, space="PSUM"))

        wp1_sb = const.tile([128, E + E + 6], BF16, tag="wp1")
        wp2_sb = const.tile([128, E + E + 128], BF16, tag="wp2")
        # wk plane first so the first K proj only waits ~0.7us of weight DMA
        nc.scalar.dma_start(wp1_sb[:, 0:E], wp1[:, 0:E])
        nc.scalar.dma_start(wp1_sb[:, E:2 * E + 6], wp1[:, E:2 * E + 6])
        # fp8 hi/lo weight planes live in the bf16-typed pack; bitcast views.
        # Weight values are pre-scaled x32 on host (fp8 subnormal floor); the
        # bias step multiplies PSUM by 1/32.
        wk_sb = wp1_sb[:, 0:E].bitcast(FP8).rearrange(
            "p (hl ec c) -> p hl ec c", hl=2, c=128)
        wq_sb = wp1_sb[:, E:2 * E].bitcast(FP8).rearrange(
            "p (hl ec c) -> p hl ec c", hl=2, c=128)
        # f32 bias bytes live in two bf16 slots each; reinterpret in place
        bk_sb = wp1_sb[:, 2 * E:2 * E + 2].bitcast(F32)
        bq_sb = wp1_sb[:, 2 * E + 2:2 * E + 4].bitcast(F32)
        bv_sb = wp1_sb[:, 2 * E + 4:2 * E + 6].bitcast(F32)
        wv_sb = wp2_sb[:, 0:E].bitcast(FP8).rearrange(
            "p (hl ec c) -> p hl ec c", hl=2, c=128)
        wo_sb = wp2_sb[:, E:2 * E]
        id_sb = wp2_sb[:, 2 * E:2 * E + 128]

        qt_sb = persist.tile([128, QC, 512], BF16, tag="qt", name="qt")
        kt_sb = [persist.tile([128, SKV], BF16, tag=f"kt{b}", name=f"kt{b}")
                 for b in range(B)]
        v_sb = [persist.tile([128, KVC_B, 130], BF16, tag=f"v{b}",
                             name=f"v{b}") for b in range(B)]
        at_sb = [persist.tile([128, 8, 128], BF16, tag=f"at{b}",
                              name=f"at{b}") for b in range(B)]
        att_T = [persist.tile([128, SQ], BF16, tag=f"aT{b}", name=f"aT{b}")
                 for b in range(B)]
        # softmax-denominator ones columns (cols 64 and 129 of each kv block)
        for b in range(B):
            nc.gpsimd.memset(v_sb[b][:, :, 64::65], 1.0)

        xq = {}     # qc -> xload tile
        xkv = {}    # (b, j) -> xload tile
        qps = {}
        kps = {}
        vps = {}
        vdt = {}    # (b, j) -> [128, 512] bf16 V in [d, kv] orientation
        vstg = {}   # (b, kc) -> [128, 128] staging for transposed V block

        fillers = deque()
        passed_markers = set()

        def drain(n):
            done = 0
            while done < n and fillers:
                u = fillers.popleft()
                if isinstance(u, tuple) and u[0] == "m":
                    passed_markers.add(u[1])
                    continue
                u()
                done += 1

        def flush_until(key):
            # force-drain the filler FIFO until `key`'s marker has passed:
            # structural emission-order barrier (e.g. all v_sb copies of a
            # batch before that batch's first attnv matmul chain is emitted)
            while key not in passed_markers:
                assert fillers, f"marker {key} never queued"
                u = fillers.popleft()
                if isinstance(u, tuple) and u[0] == "m":
                    passed_markers.add(u[1])
                    continue
                u()

        def load_x1(qc):
            xt = xload.tile([128, 2, EC, 512], FP8, tag="x", name=f"xq{qc}")
            nc.sync.dma_start(xt[:], x1t[qc])
            xq[qc] = xt

        def load_x2(b, j, split=False):
            xt = xload.tile([128, 2, EC, 512], FP8, tag="x",
                            name=f"xkv{b}_{j}")
            if split:
                nc.sync.dma_start(xt[:, :, 0:4, :], x2t[b * 4 + j][:, :, 0:4, :])
                nc.sync.dma_start(xt[:, :, 4:8, :], x2t[b * 4 + j][:, :, 4:8, :])
            else:
                nc.sync.dma_start(xt[:], x2t[b * 4 + j])
            xkv[(b, j)] = xt

        # hi/lo fp8 DoubleRow projection: x@W ~ xhi@Whi + xlo@Whi + xhi@Wlo
        # (lo*lo dropped), each DR matmul covers 2 contraction chunks.
        HL = ((0, 0), (1, 0), (0, 1))   # (x plane, w plane)

        def proj_dr(psum, w4, xt, cols, cps, last):
            for i, cp in enumerate(cps):
                for k, (xhl, whl) in enumerate(HL):
                    nc.tensor.matmul(
                        psum, w4[:, whl, cp:cp + 2, :],
                        xt[:, xhl, cp:cp + 2, cols],
                        start=(cp == 0 and k == 0),
                        stop=(last and i == len(cps) - 1 and k == len(HL) - 1),
                        perf_mode=DR)

        def proj_q_mm(qc, half):
            if half == 0:
                qps[qc] = ps_pj.tile([128, 512], F32, tag="pj", name=f"qps{qc}")
            proj_dr(qps[qc][:], wq_sb, xq[qc], slice(0, 512),
                    (0, 2) if half == 0 else (4, 6), half == 1)

        def proj_q_bias(qc):
            nc.vector.tensor_scalar(qt_sb[:, qc, :], qps[qc][:], 1.0 / 32,
                                    bq_sb[:], mybir.AluOpType.mult,
                                    mybir.AluOpType.add)

        def proj_k_mm(b, j, half):
            if half == 0:
                kps[(b, j)] = ps_pj.tile([128, 512], F32, tag="pj",
                                         name=f"kps{b}_{j}")
            proj_dr(kps[(b, j)][:], wk_sb, xkv[(b, j)], slice(0, 512),
                    (0, 2) if half == 0 else (4, 6), half == 1)

        def proj_k_bias(b, j):
            nc.vector.tensor_scalar(kt_sb[b][:, j * 512:(j + 1) * 512],
                                    kps[(b, j)][:], 1.0 / 32, bk_sb[:],
                                    mybir.AluOpType.mult, mybir.AluOpType.add)

        def proj_v_mm(b, j, half):
            # normal orientation, same as K: out = [128 d, 512 kv] in PSUM
            if half == 0:
                vps[(b, j)] = ps_pj.tile([128, 512], F32, tag="pj",
                                         name=f"vps{b}_{j}")
            proj_dr(vps[(b, j)][:], wv_sb, xkv[(b, j)], slice(0, 512),
                    (0, 2) if half == 0 else (4, 6), half == 1)

        def proj_v_bias(b, j):
            t = work.tile([128, 512], BF16, tag="vdt", bufs=6,
                          name=f"vdt{b}_{j}")
            nc.vector.tensor_scalar(t[:], vps[(b, j)][:], 1.0 / 32, bv_sb[:],
                                    mybir.AluOpType.mult, mybir.AluOpType.add)
            vdt[(b, j)] = t

        def v_transp(b, j, t):
            # [d, kv] 128-block -> [kv, d] staging via transposing DMA
            kc = j * 4 + t
            if V_DMA_T:
                # ACT's HWDGE queue: the SP queue's 8 in-flight DMA sem
                # slots are needed for the x-slab prefetch stream
                s = work.tile([128, 128], BF16, tag="vstg", bufs=18,
                              name=f"vstg{b}_{kc}")
                nc.scalar.dma_start_transpose(
                    s[:], vdt[(b, j)][:, t * 128:(t + 1) * 128])
            else:
                s = ps_pj.tile([128, 128], BF16, tag="pj", name=f"vtp{b}_{kc}")
                nc.tensor.transpose(s[:], vdt[(b, j)][:, t * 128:(t + 1) * 128],
                                    id_sb[:])
            vstg[(b, kc)] = s

        def v_copy(b, kc):
            # staging -> 65-stride [V_h0|1|V_h1|1] layout (2x-mode DVE copy)
            dst = v_sb[b][:, kc].rearrange("p (h x) -> p h x", h=2)
            nc.vector.tensor_copy(dst[:, :, 0:64],
                                  vstg.pop((b, kc))[:].rearrange(
                                      "p (h x) -> p h x", h=2))

        def oproj_mm(b, g, o):
            # b < 3: two 512-col halves land in one [128,1024] ysb tile
            key = (b, o)
            yp = ps_pj.tile([128, 512], F32, tag="pj", name=f"yps{b}_{g}_{o}")
            nc.tensor.matmul(yp[:], wo_sb[:, o * 128:(o + 1) * 128],
                             att_T[b][:, g * 512:(g + 1) * 512],
                             start=True, stop=True)
            if g == 0:
                ysb[key] = work.tile([128, 1024], BF16, tag="y", bufs=5,
                                     name=f"ysb{b}_{o}")
            nc.vector.tensor_copy(ysb[key][:, g * 512:(g + 1) * 512], yp[:])

        def oproj_store(b, o):
            nc.sync.dma_start(yt_r[:, o, b * SQ:(b + 1) * SQ],
                              ysb.pop((b, o))[:])

        def oproj_tail(b, g, o):
            # b = 3 tail: per-(g,o) stores on the ACT queue; copies alternate
            # DVE/ACT so the idle post-exp ACT engine helps drain
            yp = ps_pj.tile([128, 512], F32, tag="pj", name=f"yps{b}_{g}_{o}")
            nc.tensor.matmul(yp[:], wo_sb[:, o * 128:(o + 1) * 128],
                             att_T[b][:, g * 512:(g + 1) * 512],
                             start=True, stop=True)
            yb = work.tile([128, 512], BF16, tag="yt", bufs=6,
                           name=f"ytl{b}_{g}_{o}")
            # tail runs after the last exp: ACT engine is idle, DVE still
            # has the per-qb recip/normalize chain -- copies go to ACT
            nc.scalar.copy(yb[:], yp[:])
            nc.scalar.dma_start(
                yt_r[:, o, b * SQ + g * 512: b * SQ + (g + 1) * 512], yb[:])

        ysb = {}

        def push_qproj(qc, load=True):
            out = []
            if load:
                out.append(lambda: load_x1(qc))
            out.append(lambda: proj_q_mm(qc, 0))
            out.append(lambda: (proj_q_mm(qc, 1), proj_q_bias(qc)))
            return out

        def push_kproj(b, js=range(4), load=True):
            out = []
            for j in js:
                if load:
                    out.append(lambda b=b, j=j: load_x2(b, j))
                out.append(lambda b=b, j=j: proj_k_mm(b, j, 0))
                out.append(lambda b=b, j=j: (proj_k_mm(b, j, 1),
                                             proj_k_bias(b, j)))
            return out

        def push_vproj(b, js=range(4)):
            # matmuls+bias first; then transpose/copy unit pairs kept
            # adjacent so at most two staging tiles occupy the 2-deep pj
            # PSUM ring at a time (PE-transpose path).
            out = []
            for j in js:
                out.append(lambda b=b, j=j: proj_v_mm(b, j, 0))
                out.append(lambda b=b, j=j: (proj_v_mm(b, j, 1),
                                             proj_v_bias(b, j)))
            if V_DMA_T:
                for j in js:
                    out.append(lambda b=b, j=j: (v_transp(b, j, 0),
                                                 v_transp(b, j, 1)))
                    out.append(lambda b=b, j=j: (v_transp(b, j, 2),
                                                 v_transp(b, j, 3)))
                for j in js:
                    out.append(lambda b=b, j=j: (v_copy(b, j * 4 + 0),
                                                 v_copy(b, j * 4 + 1)))
                    out.append(lambda b=b, j=j: (v_copy(b, j * 4 + 2),
                                                 v_copy(b, j * 4 + 3)))
            else:
                for j in js:
                    for t in (0, 1, 2, 3):
                        out.append(lambda b=b, j=j, t=t: (
                            v_transp(b, j, t), v_copy(b, j * 4 + t)))
            return out

        def att_transp(b, qb):
            nc.sync.dma_start_transpose(
                att_T[b][:, qb * 128:(qb + 1) * 128], at_sb[b][:, qb, :])

        def queue_oproj(b):
            # appended by attnv_steps(b,1)'s last step, AFTER every att_T
            # transpose of batch b is emitted.  Per-o interleaving keeps at
            # most ~4 ysb tiles live (5-deep ring); stores trail their
            # copies by two o-units so the SP queue never parks long.
            for o in range(EC):
                fillers.append(lambda b=b, o=o: oproj_mm(b, 0, o))
                fillers.append(lambda b=b, o=o: oproj_mm(b, 1, o))
                if o >= 2:
                    fillers.append(lambda b=b, o=o: oproj_store(b, o - 2))
            for o in (EC - 2, EC - 1):
                fillers.append(lambda b=b, o=o: oproj_store(b, o))

        def interleave(*lists):
            # round-robin so slow-consumer thunks never cluster on the 2-deep
            # pj PSUM ring
            lists = [list(x) for x in lists if x]
            while lists:
                for x in list(lists):
                    fillers.append(x.pop(0))
                    if not x:
                        lists.remove(x)

        pts = {}

        def scores_steps(b, h, u_split=False):
            pt = ptp.tile([128, KVC_B, SQ], BF16, tag="pt", name=f"pt{b}_{h}")
            pts[(b, h)] = pt
            if u_split:
                # startup window: per-u halves grouped by x1-slab arrival so
                # the exp stream starts as soon as qc0 lands
                for u in range(2):
                    for kc in range(KVC_B):
                        sp = ps_s.tile([128, SQ], F32, tag="s",
                                       name=f"sps{b}_{h}_{kc}_{u}")
                        nc.tensor.matmul(
                            sp[:, 0:512],
                            kt_sb[b][h * 64:h * 64 + 64,
                                     kc * 128:(kc + 1) * 128],
                            qt_sb[h * 64:h * 64 + 64, 2 * b + u, :],
                            start=True, stop=True)
                        nc.scalar.activation(
                            pt[:, kc, u * 512:(u + 1) * 512], sp[:, 0:512],
                            Exp, bias=-SHIFT, scale=0.125)
                        if kc % 2 == 1:
                            yield
                return
            for kc in range(KVC_B):
                sp = ps_s.tile([128, SQ], F32, tag="s",
                               name=f"sps{b}_{h}_{kc}")
                for u in range(2):
                    nc.tensor.matmul(
                        sp[:, u * 512:(u + 1) * 512],
                        kt_sb[b][h * 64:h * 64 + 64,
                                 kc * 128:(kc + 1) * 128],
                        qt_sb[h * 64:h * 64 + 64, 2 * b + u, :],
                        start=True, stop=True)
                nc.scalar.activation(pt[:, kc, :], sp[:], Exp,
                                     bias=-SHIFT, scale=0.125)
                yield

        def attnv_steps(b, h):
            pt = pts[(b, h)]
            for qb in range(8):
                op = ps_o.tile([128, 65], F32, tag="o", name=f"o{b}_{h}_{qb}")
                for kc2 in range(KVC_B):
                    nc.tensor.matmul(
                        op[:], pt[:, kc2, qb * 128:(qb + 1) * 128],
                        v_sb[b][:, kc2, h * 65:h * 65 + 65],
                        start=(kc2 == 0), stop=(kc2 == KVC_B - 1))
                rc = work.tile([128, 1], F32, tag="rc", bufs=6,
                               name=f"rc{b}_{h}_{qb}")
                nc.vector.reciprocal(rc[:], op[:, 64:65])
                nc.vector.tensor_scalar_mul(at_sb[b][:, qb, h * 64:h * 64 + 64],
                                            op[:, 0:64], rc[:])
                if h == 1:
                    if b < B - 1 and ATT_DMA_T:
                        # transposing DMA, emitted two qb-steps behind its
                        # normalize so the in-order SP queue never parks on
                        # an unsatisfied wait; the final step catches up.
                        if qb >= 2:
                            att_transp(b, qb - 2)
                        if qb == 7:
                            att_transp(b, 6)
                            att_transp(b, 7)
                            queue_oproj(b)
                    elif b < B - 1:
                        tp = ps_pj.tile([128, 128], BF16, tag="pj",
                                        name=f"tp{b}_{qb}")
                        nc.tensor.transpose(tp[:], at_sb[b][:, qb, :], id_sb[:])
                        nc.vector.tensor_copy(
                            att_T[b][:, qb * 128:(qb + 1) * 128], tp[:])
                        if qb == 7:
                            queue_oproj(b)
                    else:
                        # tail batch: low-latency PE transpose path; the
                        # att_T copy rides the idle post-exp ACT engine
                        tp = ps_pj.tile([128, 128], BF16, tag="pj",
                                        name=f"tp{b}_{qb}")
                        nc.tensor.transpose(tp[:], at_sb[b][:, qb, :], id_sb[:])
                        nc.scalar.copy(
                            att_T[b][:, qb * 128:(qb + 1) * 128], tp[:])
                        if 3 <= qb < 7:
                            # spread g0 out-proj units over qb 3-6
                            for o in (2 * (qb - 3), 2 * (qb - 3) + 1):
                                oproj_tail(b, 0, o)
                        elif qb == 7:
                            for o in range(EC):
                                oproj_tail(b, 1, o)
                yield

        def drive(s, a_old, n_old, a_new, ds=2):
            # interleave the current window's scores/exp stream with the
            # previous window's attn@V stream.  The last TWO attnV steps are
            # carried past the window boundary and flushed one-per-step right
            # after the next window's first scores steps, so the boundary exp
            # never queues behind them.  The lag is FIXED at two steps
            # (consume 6 new + flush 2 old = produce 8 per window), so
            # nothing older than the immediately-previous window is ever
            # pending when a window's scores start writing the pt ring.
            k = 0
            acount = 0
            while s is not None:
                try:
                    next(s)
                    k += 1
                    drain(ds)
                except StopIteration:
                    s = None
                if n_old > 0:
                    try:
                        next(a_old)
                        drain(1)
                    except StopIteration:
                        pass
                    n_old -= 1
                if a_new is not None and k % 2 == 0 and acount < 6:
                    try:
                        next(a_new)
                        acount += 1
                        drain(1)
                    except StopIteration:
                        a_new = None
            return a_new, (8 - acount if a_new is not None else 0)

        # ---- startup: minimal critical path to the first exp ----
        # first window runs u_split so exp starts once qc0 (x1 slab 0) lands:
        # critical path = wk + x2(0,0) + x1(0) DMA bytes only
        load_x2(0, 0, split=True)
        load_x1(0)
        proj_k_mm(0, 0, 0)
        proj_k_mm(0, 0, 1)
        proj_k_bias(0, 0)
        proj_q_mm(0, 0)
        proj_q_mm(0, 1)
        proj_q_bias(0)
        load_x1(1)
        nc.scalar.dma_start(wp2_sb[:], wp2[:])
        load_x2(0, 1)
        load_x2(0, 2)
        proj_q_mm(1, 0)
        proj_q_mm(1, 1)
        proj_q_bias(1)
        # queue for batch-0/1 windows: remaining k(0), v(0), q(2,3), then
        # kv(1), kv(2), k(3) in emission-safe order (v(b) before any later
        # load that recycles b's xload slots)
        fillers.append(lambda: load_x2(0, 3))
        for t in push_kproj(0, js=range(1, 4), load=False):
            fillers.append(t)
        interleave(push_vproj(0), push_qproj(2) + push_qproj(3))
        fillers.append(("m", ("v", 0)))
        for t in (push_kproj(1) + push_vproj(1)):
            fillers.append(t)
        fillers.append(("m", ("v", 1)))
        for t in push_kproj(2):
            fillers.append(t)

        # Filler pushes are scheduled per window.  oproj(b) may only be
        # pushed once attnv(b,1) has been fully EMITTED (it reads att_T[b]),
        # which happens during the drive of the following window.
        windows = [(b, h) for b in range(B) for h in (0, 1)]
        def push2(b):
            interleave(push_qproj(4) + push_qproj(5), push_vproj(2))
            fillers.append(("m", ("v", 2)))

        def push4(b):
            interleave(push_qproj(6) + push_qproj(7), push_vproj(3))
            fillers.append(("m", ("v", 3)))

        pushes = {
            (1, 0): push2,
            (1, 1): lambda b: interleave(push_kproj(3)),
            (2, 0): push4,
        }
        old_a, old_n = None, 0   # carried remainder of attnv(i-2)
        new_a = None             # attnv(i-1), fresh each window
        for i, (b, h) in enumerate(windows):
            if h == 1:
                # all of batch b's v_sb copies must be EMITTED before
                # attnv(b,0)'s matmul chains (driven in this window) are
                flush_until(("v", b))
            s = scores_steps(b, h, u_split=(i == 0))
            old_a, old_n = drive(s, old_a, old_n, new_a)
            new_a = attnv_steps(b, h)
            if (b, h) in pushes:
                pushes[(b, h)](b)
        for g in (old_a, new_a):
            while g is not None:
                try:
                    next(g)
                    drain(1)
                except StopIteration:
                    g = None
        while fillers:
            drain(len(fillers))

        if DBG:
            nc.sync.dma_start(dbg_qt[:], qt_sb[:])
            for b in range(B):
                nc.sync.dma_start(dbg_kt[b], kt_sb[b][:])
                nc.sync.dma_start(dbg_v[b], v_sb[b][:])
                nc.sync.dma_start(dbg_at[b], at_sb[b][:])
                nc.sync.dma_start(dbg_aT[b], att_T[b][:])
            nc.sync.dma_start(dbg_pt[:], pts[(B - 1, 1)][:])

    nc.compile()
    return nc


def _get_nc():
    if "nc" not in _CACHE:
        _CACHE["nc"] = _build()
    return _CACHE["nc"]


def _tile_x(xt2d, nchunks):
    # [E, R] -> [R/512, 128, EC, 512]
    return np.ascontiguousarray(
        xt2d.reshape(EC, 128, nchunks, 512).transpose(2, 1, 0, 3))


def _tile_w(wt_slice):
    # [E, 128] -> [128, EC, 128]
    return np.ascontiguousarray(
        wt_slice.reshape(EC, 128, 128).transpose(1, 0, 2))


def _hilo(a):
    f8 = ml_dtypes.float8_e4m3
    hi = a.astype(f8)
    lo = (a - hi.astype(np.float32)).astype(f8)
    return hi, lo


def _tile_x_hilo(xt2d, nchunks):
    # [E, R] f32 -> [R/512, 128, 2, EC, 512] fp8 (hi, lo planes)
    hi, lo = _hilo(xt2d)
    return np.ascontiguousarray(
        np.stack([_tile_x(hi, nchunks), _tile_x(lo, nchunks)], axis=2))


def make_in_maps(x1, x2, Wq, bq, Wk, bk, Wv, bv, Wo, bo=None):
    bf = ml_dtypes.bfloat16
    x1f = np.ascontiguousarray(np.asarray(x1, np.float32).reshape(Q_ROWS, E).T)
    x2f = np.ascontiguousarray(np.asarray(x2, np.float32).reshape(KV_ROWS, E).T)
    x1t = _tile_x_hilo(x1f, QC)
    x2t = _tile_x_hilo(x2f, KV_ROWS // 512)
    # weights scaled x32 so fp8 lo-planes stay above the subnormal floor
    WqT = np.asarray(Wq, dtype=np.float32).T * 32.0
    WkT = np.asarray(Wk, dtype=np.float32).T * 32.0
    WvT = np.asarray(Wv, dtype=np.float32).T * 32.0
    WoT = np.asarray(Wo, dtype=np.float32).T.astype(bf)
    ident = np.eye(128, dtype=bf)
    bqa = np.asarray(bq, np.float32)
    bka = np.asarray(bk, np.float32)
    bva = np.asarray(bv, np.float32)

    def pack_w_hilo(wT_slice):
        # -> [128, E] uint16 holding (hi[1024] | lo[1024]) fp8 bytes
        hi, lo = _hilo(wT_slice)
        buf = np.empty((128, 2 * E), np.uint8)
        buf[:, 0:E] = _tile_w(hi).reshape(128, E).view(np.uint8)
        buf[:, E:2 * E] = _tile_w(lo).reshape(128, E).view(np.uint8)
        return buf.view(np.uint16)

    in_maps = []
    for c in range(N_CORES):
        s = slice(128 * c, 128 * (c + 1))
        wp1 = np.zeros((128, 2 * E + 6), dtype=bf)
        wp1u = wp1.view(np.uint16)
        wp1u[:, 0:E] = pack_w_hilo(WkT[:, s])
        wp1u[:, E:2 * E] = pack_w_hilo(WqT[:, s])
        wp1u[:, 2 * E:2 * E + 2] = bka[s].view(np.uint16).reshape(128, 2)
        wp1u[:, 2 * E + 2:2 * E + 4] = bqa[s].view(np.uint16).reshape(128, 2)
        wp1u[:, 2 * E + 4:2 * E + 6] = bva[s].view(np.uint16).reshape(128, 2)
        wp2 = np.zeros((128, 2 * E + 128), dtype=bf)
        wp2.view(np.uint16)[:, 0:E] = pack_w_hilo(WvT[:, s])
        wp2[:, E:2 * E] = WoT[s, :]
        wp2[:, 2 * E:] = ident
        in_maps.append({
            "x1t": x1t, "x2t": x2t,
            "wp1": wp1, "wp2": wp2,
        })
    return in_maps


def kernel(x1, x2, Wq, bq, Wk, bk, Wv, bv, Wo, bo):
    nc = _get_nc()
    in_maps = make_in_maps(x1, x2, Wq, bq, Wk, bk, Wv, bv, Wo)
    res = run_bass_kernel_spmd(nc, in_maps, list(range(N_CORES)))
    ytf = res.results[0]["yt"].astype(np.float64)
    for c in range(1, N_CORES):
        ytf += res.results[c]["yt"].astype(np.float64)
    y = ytf.T.astype(np.float32) + np.asarray(bo, np.float32)[None, :]
    return y.reshape(B, SQ, E)


# revision 39
# speedup vs baseline: 1.0038x; 1.0038x over previous
"""MultiHeadCrossAttention on 8 TRN2 NeuronCores.

Sharding: tensor-parallel over heads (16 heads -> 2 per core); each core
writes a full-size partial of y.T which the host sums (replaces the
all-reduce). Design vs the f32r baseline (254us -> ~196us cost-model):

  * Activations bf16 end-to-end (tolerance is 2e-2; bf16 lands ~6e-3),
    halving DMA traffic vs f32.
  * q/k/v projections run fp8e4m3 DoubleRow (0.5 cyc/row, 2 contraction
    chunks per pass) with hi+lo splitting: x @ W ~ xhi@Whi + xlo@Whi +
    xhi@Wlo (lo*lo dropped). 12 DR matmuls replace 16 bf16-equivalents per
    512-col chunk (-25% PE) and land MORE accurate than bf16 (~1.3e-3).
    W is host-prescaled x32 so the fp8 lo plane clears the subnormal floor;
    the PSUM->SBUF bias step multiplies by 1/32. (Plain-fp8 scores/attnV/
    out-proj all FAIL the 2e-2 budget -- measured 3-6e-2 -- so everything
    else stays bf16.)
  * V projection swaps matmul roles (stationary = x2 chunk, moving = Wv) so
    V lands in PSUM already [kv, d]-oriented: no PE transposes for V.
  * attn@V reoriented: stationary = P.T [128kv, 128q] block, moving =
    [V|1] [128kv, 65] -> out [128q, 65]: 66.5k PE cycles instead of 131k,
    and the softmax denominator rides along as the ones column.
  * Normalize is per-partition (q on partitions): vector reciprocal of the
    denominator column + tensor_scalar_mul; no gpsimd broadcast.
  * attn tiles are PE-transposed back to [d, q] for the out-projection.
  * Weights/biases packed into two DMA transfers (wp1/wp2); f32 bias bytes
    and fp8 weight planes live inside the bf16 pack via bitcast views.

Schedule: exp on ACT is the pacing stream (~139us busy; ACT is the only
engine with exp, 0.83ns/row + 185ns/inst PSUM/SBUF access). Windows of 16
scores-matmul+exp steps are interleaved with the previous window's attn@V
steps, and a filler queue drips projection / out-proj / DMA work into each
step so PE (~151us busy) stays dense. PSUM budget (8 banks): scores 2x2,
attn@V out 2x1, everything else shares a 2x1 ring.

v3 deltas (188.5us -> target ~183):
  * wp1 loads in two pieces (wk plane first) and x2(0,0) in two halves;
    the first window runs u_split so the exp stream starts at ~7.6us
    instead of ~11.5us.
  * y stores merged per (b, o): two [128,512] DVE copies land in one
    [128,1024] SBUF tile and one DMA (24 y DMAs on the SP queue instead
    of 64, easing the 8-slot DMA in-flight ring).
  * b=3 tail y stores ride the ACT HWDGE queue (idle after the last exp).
"""
import numpy as np
import ml_dtypes
from collections import deque
from contextlib import ExitStack

import concourse.bass as bass
import concourse.mybir as mybir
import concourse.tile as tile
from concourse import bacc
from concourse.bass_utils import run_bass_kernel_spmd

N_CORES = 8
B, SQ, SKV, E, DH = 4, 1024, 2048, 1024, 64
Q_ROWS = B * SQ      # 4096
KV_ROWS = B * SKV    # 8192
EC = E // 128        # 8 contraction chunks
QC = Q_ROWS // 512   # 8 q column chunks
KVC_B = SKV // 128   # 16 kv blocks per batch
F32 = mybir.dt.float32
BF16 = mybir.dt.bfloat16
FP8 = mybir.dt.float8e4
DR = mybir.MatmulPerfMode.DoubleRow
Exp = mybir.ActivationFunctionType.Exp
SHIFT = 0.0

_CACHE = {}


def _build():
    nc = bacc.Bacc("TRN2", target_bir_lowering=False, debug=False,
                   num_devices=N_CORES)
    # x slabs as fp8 hi/lo pairs (same bytes as bf16, but projections can run
    # DoubleRow: 2 contraction chunks per pass at 0.5 cyc/row)
    x1t = nc.dram_tensor("x1t", [QC, 128, 2, EC, 512], FP8,
                         kind="ExternalInput").ap()
    x2t = nc.dram_tensor("x2t", [KV_ROWS // 512, 128, 2, EC, 512], FP8,
                         kind="ExternalInput").ap()
    # packed weights: wk first so the first K proj waits ~0.7us of weights
    # wp1 = [Wk hi|lo fp8 | Wq hi|lo fp8 | bk | bq | bv-row(row0)]
    # wp2 = [Wv hi|lo fp8 | Wo.T bf16 | identity bf16]
    wp1 = nc.dram_tensor("wp1", [128, E + E + 4 + 128], BF16,
                         kind="ExternalInput").ap()
    wp2 = nc.dram_tensor("wp2", [128, E + E + 128], BF16,
                         kind="ExternalInput").ap()
    yt = nc.dram_tensor("yt", [E, Q_ROWS], BF16, kind="ExternalOutput").ap()
    yt_r = yt.rearrange("(oc p) q -> p oc q", p=128)

    with tile.TileContext(nc) as tc, ExitStack() as ctx:
        const = ctx.enter_context(tc.tile_pool(name="const", bufs=1))
        persist = ctx.enter_context(tc.tile_pool(name="persist", bufs=1))
        ptp = ctx.enter_context(tc.tile_pool(name="ptp", bufs=2))
        xload = ctx.enter_context(tc.tile_pool(name="xload", bufs=6))
        work = ctx.enter_context(tc.tile_pool(name="work", bufs=3))
        ps_pj = ctx.enter_context(tc.tile_pool(name="ps_pj", bufs=2, space="PSUM"))
        ps_s = ctx.enter_context(tc.tile_pool(name="ps_s", bufs=2, space="PSUM"))
        ps_o = ctx.enter_context(tc.tile_pool(name="ps_o", bufs=2, space="PSUM"))

        wp1_sb = const.tile([128, E + E + 4 + 128], BF16, tag="wp1")
        wp2_sb = const.tile([128, E + E + 128], BF16, tag="wp2")
        bv_row = const.tile([128, 128], BF16, tag="bvrow")
        # first packed-weight pieces go through the ACT DGE queue so the SP
        # queue starts on the big x-slab loads immediately; wk plane first
        nc.scalar.dma_start(wp1_sb[:, 0:E], wp1[:, 0:E])
        nc.scalar.dma_start(wp1_sb[:, E:2 * E + 4 + 128],
                            wp1[:, E:2 * E + 4 + 128])
        # fp8 hi/lo weight planes live in the bf16-typed pack; bitcast views.
        # Weight values are pre-scaled x32 on host (fp8 subnormal floor); the
        # bias step multiplies PSUM by 1/32.
        wk_sb = wp1_sb[:, 0:E].bitcast(FP8).rearrange(
            "p (hl ec c) -> p hl ec c", hl=2, c=128)
        wq_sb = wp1_sb[:, E:2 * E].bitcast(FP8).rearrange(
            "p (hl ec c) -> p hl ec c", hl=2, c=128)
        # f32 bias bytes live in two bf16 slots each; reinterpret in place
        bk_sb = wp1_sb[:, 2 * E:2 * E + 2].bitcast(F32)
        bq_sb = wp1_sb[:, 2 * E + 2:2 * E + 4].bitcast(F32)
        bvr_sb = wp1_sb[0:1, 2 * E + 4:2 * E + 4 + 128]
        wv_sb = wp2_sb[:, 0:E].bitcast(FP8).rearrange(
            "p (hl ec c) -> p hl ec c", hl=2, c=128)
        wo_sb = wp2_sb[:, E:2 * E]
        id_sb = wp2_sb[:, 2 * E:2 * E + 128]
        nc.gpsimd.partition_broadcast(bv_row[:], bvr_sb[:])

        qt_sb = persist.tile([128, QC, 512], BF16, tag="qt", name="qt")
        kt_sb = [persist.tile([128, SKV], BF16, tag=f"kt{b}", name=f"kt{b}")
                 for b in range(B)]
        v_sb = [persist.tile([128, KVC_B, 130], BF16, tag=f"v{b}",
                             name=f"v{b}") for b in range(B)]
        at_sb = [persist.tile([128, 8, 128], BF16, tag=f"at{b}",
                              name=f"at{b}") for b in range(B)]
        att_T = [persist.tile([128, SQ], BF16, tag=f"aT{b}", name=f"aT{b}")
                 for b in range(B)]
        # softmax-denominator ones columns (cols 64 and 129 of each kv block)
        for b in range(B):
            nc.gpsimd.memset(v_sb[b][:, :, 64::65], 1.0)

        xq = {}     # qc -> xload tile
        xkv = {}    # (b, j) -> xload tile
        qps = {}
        kps = {}
        vps = {}
        ysb = {}    # (b, o) -> merged [128, 1024] y tile

        fillers = deque()

        def drain(n):
            for _ in range(min(n, len(fillers))):
                fillers.popleft()()

        def load_x1(qc):
            xt = xload.tile([128, 2, EC, 512], FP8, tag="x", name=f"xq{qc}")
            nc.sync.dma_start(xt[:], x1t[qc])
            xq[qc] = xt

        def load_x2(b, j, split=False):
            xt = xload.tile([128, 2, EC, 512], FP8, tag="x",
                            name=f"xkv{b}_{j}")
            if split:
                nc.sync.dma_start(xt[:, :, 0:4, :], x2t[b * 4 + j][:, :, 0:4, :])
                nc.sync.dma_start(xt[:, :, 4:8, :], x2t[b * 4 + j][:, :, 4:8, :])
            else:
                nc.sync.dma_start(xt[:], x2t[b * 4 + j])
            xkv[(b, j)] = xt

        # hi/lo fp8 DoubleRow projection: x@W ~ xhi@Whi + xlo@Whi + xhi@Wlo
        # (lo*lo dropped), each DR matmul covers 2 contraction chunks.
        HL = ((0, 0), (1, 0), (0, 1))   # (x plane, w plane)

        def proj_dr(psum, w4, xt, cols, cps, last):
            for i, cp in enumerate(cps):
                for k, (xhl, whl) in enumerate(HL):
                    nc.tensor.matmul(
                        psum, w4[:, whl, cp:cp + 2, :],
                        xt[:, xhl, cp:cp + 2, cols],
                        start=(cp == 0 and k == 0),
                        stop=(last and i == len(cps) - 1 and k == len(HL) - 1),
                        perf_mode=DR)

        def proj_q_mm(qc, half):
            if half == 0:
                qps[qc] = ps_pj.tile([128, 512], F32, tag="pj", name=f"qps{qc}")
            proj_dr(qps[qc][:], wq_sb, xq[qc], slice(0, 512),
                    (0, 2) if half == 0 else (4, 6), half == 1)

        def proj_q_bias(qc):
            nc.vector.tensor_scalar(qt_sb[:, qc, :], qps[qc][:], 1.0 / 32,
                                    bq_sb[:], mybir.AluOpType.mult,
                                    mybir.AluOpType.add)

        def proj_k_mm(b, j, half):
            if half == 0:
                kps[(b, j)] = ps_pj.tile([128, 512], F32, tag="pj",
                                         name=f"kps{b}_{j}")
            proj_dr(kps[(b, j)][:], wk_sb, xkv[(b, j)], slice(0, 512),
                    (0, 2) if half == 0 else (4, 6), half == 1)

        def proj_k_bias(b, j):
            nc.vector.tensor_scalar(kt_sb[b][:, j * 512:(j + 1) * 512],
                                    kps[(b, j)][:], 1.0 / 32, bk_sb[:],
                                    mybir.AluOpType.mult, mybir.AluOpType.add)

        def proj_v_blk(b, j, t):
            # swapped-role projection: stationary = x2 chunk, moving = Wv
            # -> V comes out of PSUM already [kv, d]; no transpose needed
            kc = j * 4 + t
            vp = ps_pj.tile([128, 128], F32, tag="pj", name=f"vps{b}_{kc}")
            cols = slice(t * 128, (t + 1) * 128)
            for cp in (0, 2, 4, 6):
                for k, (xhl, whl) in enumerate(HL):
                    nc.tensor.matmul(
                        vp[:], xkv[(b, j)][:, xhl, cp:cp + 2, cols],
                        wv_sb[:, whl, cp:cp + 2, :],
                        start=(cp == 0 and k == 0),
                        stop=(cp == 6 and k == len(HL) - 1),
                        perf_mode=DR)
            dst = v_sb[b][:, kc].rearrange("p (h x) -> p h x", h=2)
            r2 = "p (h x) -> p h x"
            nc.vector.scalar_tensor_tensor(
                dst[:, :, 0:64], vp[:].rearrange(r2, h=2), 1.0 / 32,
                bv_row[:].rearrange(r2, h=2),
                mybir.AluOpType.mult, mybir.AluOpType.add)

        def oproj_o(b, g, o):
            yp = ps_pj.tile([128, 512], F32, tag="pj", name=f"yps{b}_{g}_{o}")
            nc.tensor.matmul(yp[:], wo_sb[:, o * 128:(o + 1) * 128],
                             att_T[b][:, g * 512:(g + 1) * 512],
                             start=True, stop=True)
            if b == B - 1:
                # tail: per-(g,o) stores on the idle ACT queue; copies
                # alternate DVE/ACT so neither queue head-of-line blocks
                yb = work.tile([128, 512], BF16, tag="yt", bufs=6,
                               name=f"ytl{b}_{g}_{o}")
                if o % 2 == 0:
                    nc.scalar.copy(yb[:], yp[:])
                else:
                    nc.vector.tensor_copy(yb[:], yp[:])
                nc.scalar.dma_start(
                    yt_r[:, o, b * SQ + g * 512: b * SQ + (g + 1) * 512],
                    yb[:])
                return
            if g == 0:
                ysb[(b, o)] = work.tile([128, 1024], BF16, tag="y", bufs=9,
                                        name=f"ysb{b}_{o}")
            nc.vector.tensor_copy(ysb[(b, o)][:, g * 512:(g + 1) * 512], yp[:])
            if g == 1:
                nc.sync.dma_start(yt_r[:, o, b * SQ:(b + 1) * SQ],
                                  ysb.pop((b, o))[:])

        def push_qproj(qc, load=True):
            out = []
            if load:
                out.append(lambda: load_x1(qc))
            out.append(lambda: proj_q_mm(qc, 0))
            out.append(lambda: (proj_q_mm(qc, 1), proj_q_bias(qc)))
            return out

        def push_kproj(b, js=range(4), load=True):
            out = []
            for j in js:
                if load:
                    out.append(lambda b=b, j=j: load_x2(b, j))
                out.append(lambda b=b, j=j: proj_k_mm(b, j, 0))
                out.append(lambda b=b, j=j: (proj_k_mm(b, j, 1),
                                             proj_k_bias(b, j)))
            return out

        def push_vproj(b):
            return [lambda b=b, j=j, t=t: proj_v_blk(b, j, t)
                    for j in range(4) for t in range(4)]

        def push_oproj(b, gs=(0, 1)):
            return [lambda b=b, g=g, o=o: oproj_o(b, g, o)
                    for g in gs for o in range(EC)]

        def interleave(*lists):
            # round-robin so slow-consumer thunks (oproj) never cluster on
            # the 2-deep pj PSUM ring
            lists = [list(x) for x in lists if x]
            while lists:
                for x in list(lists):
                    fillers.append(x.pop(0))
                    if not x:
                        lists.remove(x)

        pts = {}

        def scores_steps(b, h, u_split=False):
            pt = ptp.tile([128, KVC_B, SQ], BF16, tag="pt", name=f"pt{b}_{h}")
            pts[(b, h)] = pt
            if u_split:
                # startup window: per-u halves grouped by x2-slab arrival so
                # exp tracks the DMA landings as closely as possible
                for j in range(4):
                    for u in range(2):
                        for kc in range(4 * j, 4 * j + 4):
                            sp = ps_s.tile([128, 512], F32, tag="s",
                                           name=f"sps{b}_{h}_{kc}_{u}")
                            nc.tensor.matmul(
                                sp[:],
                                kt_sb[b][h * 64:h * 64 + 64,
                                         kc * 128:(kc + 1) * 128],
                                qt_sb[h * 64:h * 64 + 64, 2 * b + u, :],
                                start=True, stop=True)
                            nc.scalar.activation(
                                pt[:, kc, u * 512:(u + 1) * 512], sp[:], Exp,
                                bias=-SHIFT, scale=0.125)
                            yield
            else:
                for kc in range(KVC_B):
                    sp = ps_s.tile([128, SQ], F32, tag="s",
                                   name=f"sps{b}_{h}_{kc}")
                    for u in range(2):
                        nc.tensor.matmul(
                            sp[:, u * 512:(u + 1) * 512],
                            kt_sb[b][h * 64:h * 64 + 64,
                                     kc * 128:(kc + 1) * 128],
                            qt_sb[h * 64:h * 64 + 64, 2 * b + u, :],
                            start=True, stop=True)
                    nc.scalar.activation(pt[:, kc, :], sp[:], Exp,
                                         bias=-SHIFT, scale=0.125)
                    yield

        def attnv_steps(b, h):
            pt = pts[(b, h)]
            for qb in range(8):
                op = ps_o.tile([128, 65], F32, tag="o", name=f"o{b}_{h}_{qb}")
                for kc2 in range(KVC_B):
                    nc.tensor.matmul(
                        op[:], pt[:, kc2, qb * 128:(qb + 1) * 128],
                        v_sb[b][:, kc2, h * 65:h * 65 + 65],
                        start=(kc2 == 0), stop=(kc2 == KVC_B - 1))
                rc = work.tile([128, 1], F32, tag="rc", bufs=6,
                               name=f"rc{b}_{h}_{qb}")
                nc.vector.reciprocal(rc[:], op[:, 64:65])
                nc.vector.tensor_scalar_mul(at_sb[b][:, qb, h * 64:h * 64 + 64],
                                            op[:, 0:64], rc[:])
                if h == 1:
                    tp = ps_pj.tile([128, 128], BF16, tag="pj",
                                    name=f"tp{b}_{qb}")
                    nc.tensor.transpose(tp[:], at_sb[b][:, qb, :], id_sb[:])
                    nc.vector.tensor_copy(att_T[b][:, qb * 128:(qb + 1) * 128],
                                          tp[:])
                    if b == B - 1 and 3 <= qb < 7:
                        # spread g0 out-proj units over qb 3-6
                        for o in (2 * (qb - 3), 2 * (qb - 3) + 1):
                            oproj_o(b, 0, o)
                    elif b == B - 1 and qb == 7:
                        for o in range(EC):
                            oproj_o(b, 1, o)
                yield

        def drive(s, a_old, n_old, a_new, ds=2):
            # interleave the current window's scores/exp stream with the
            # previous window's attn@V stream.  The last TWO attnV steps are
            # carried past the window boundary and flushed one-per-step right
            # after the next window's first scores steps, so the boundary exp
            # never queues behind them.  The lag is FIXED at two steps
            # (consume 6 new + flush 2 old = produce 8 per window), so
            # nothing older than the immediately-previous window is ever
            # pending when a window's scores start writing the pt ring.
            k = 0
            acount = 0
            while s is not None:
                try:
                    next(s)
                    k += 1
                    drain(ds)
                except StopIteration:
                    s = None
                if n_old > 0:
                    try:
                        next(a_old)
                        drain(1)
                    except StopIteration:
                        pass
                    n_old -= 1
                if a_new is not None and k % 2 == 0 and acount < 6:
                    try:
                        next(a_new)
                        acount += 1
                        drain(1)
                    except StopIteration:
                        a_new = None
            return a_new, (8 - acount if a_new is not None else 0)

        # ---- startup: minimal critical path to the first exp ----
        # window 0 runs u_split grouped by x2-slab arrival: first exp needs
        # only wk + x2(0,0) + x1(0) DMA bytes (~7.6us)
        load_x2(0, 0, split=True)
        load_x1(0)
        proj_k_mm(0, 0, 0)
        proj_k_mm(0, 0, 1)
        proj_k_bias(0, 0)
        proj_q_mm(0, 0)
        proj_q_mm(0, 1)
        proj_q_bias(0)
        load_x1(1)
        load_x2(0, 1)
        nc.scalar.dma_start(wp2_sb[:], wp2[:])
        load_x2(0, 2)
        load_x2(0, 3)
        proj_q_mm(1, 0)
        proj_q_mm(1, 1)
        proj_q_bias(1)
        # queue for batch-0/1 windows: remaining k(0), v(0), q(2,3), then
        # kv(1), kv(2), k(3) in emission-safe order (v(b) before any later
        # load that recycles b's xload slots)
        for t in push_kproj(0, js=range(1, 4), load=False):
            fillers.append(t)
        interleave(push_vproj(0), push_qproj(2) + push_qproj(3))
        for t in (push_kproj(1) + push_vproj(1) + push_kproj(2)):
            fillers.append(t)

        # Filler pushes are scheduled per window.  oproj(b) may only be
        # pushed once attnv(b,1) has been fully EMITTED (it reads att_T[b]),
        # which happens during the drive of the following window.
        windows = [(b, h) for b in range(B) for h in (0, 1)]
        pushes = {
            (1, 0): lambda: interleave(
                push_oproj(0),
                push_qproj(4) + push_qproj(5) + push_vproj(2)),
            (1, 1): lambda: interleave(push_kproj(3)),
            (2, 0): lambda: interleave(
                push_oproj(1),
                push_qproj(6) + push_qproj(7) + push_vproj(3)),
            (3, 0): lambda: interleave(push_oproj(2)),
        }
        old_a, old_n = None, 0   # carried remainder of attnv(i-2)
        new_a = None             # attnv(i-1), fresh each window
        for i, (b, h) in enumerate(windows):
            s = scores_steps(b, h, u_split=(i == 0))
            old_a, old_n = drive(s, old_a, old_n, new_a)
            new_a = attnv_steps(b, h)
            if (b, h) in pushes:
                pushes[(b, h)]()
        for g in (old_a, new_a):
            while g is not None:
                try:
                    next(g)
                    drain(1)
                except StopIteration:
                    g = None
        while fillers:
            drain(len(fillers))

    nc.compile()
    return nc


def _get_nc():
    if "nc" not in _CACHE:
        _CACHE["nc"] = _build()
    return _CACHE["nc"]


def _tile_x(xt2d, nchunks):
    # [E, R] -> [R/512, 128, EC, 512]
    return np.ascontiguousarray(
        xt2d.reshape(EC, 128, nchunks, 512).transpose(2, 1, 0, 3))


def _tile_w(wt_slice):
    # [E, 128] -> [128, EC, 128]
    return np.ascontiguousarray(
        wt_slice.reshape(EC, 128, 128).transpose(1, 0, 2))


def _hilo(a):
    f8 = ml_dtypes.float8_e4m3
    hi = a.astype(f8)
    lo = (a - hi.astype(np.float32)).astype(f8)
    return hi, lo


def _tile_x_hilo(xt2d, nchunks):
    # [E, R] f32 -> [R/512, 128, 2, EC, 512] fp8 (hi, lo planes)
    hi, lo = _hilo(xt2d)
    return np.ascontiguousarray(
        np.stack([_tile_x(hi, nchunks), _tile_x(lo, nchunks)], axis=2))


def make_in_maps(x1, x2, Wq, bq, Wk, bk, Wv, bv, Wo, bo=None):
    bf = ml_dtypes.bfloat16
    x1f = np.ascontiguousarray(np.asarray(x1, np.float32).reshape(Q_ROWS, E).T)
    x2f = np.ascontiguousarray(np.asarray(x2, np.float32).reshape(KV_ROWS, E).T)
    x1t = _tile_x_hilo(x1f, QC)
    x2t = _tile_x_hilo(x2f, KV_ROWS // 512)
    # weights scaled x32 so fp8 lo-planes stay above the subnormal floor
    WqT = np.asarray(Wq, dtype=np.float32).T * 32.0
    WkT = np.asarray(Wk, dtype=np.float32).T * 32.0
    WvT = np.asarray(Wv, dtype=np.float32).T * 32.0
    WoT = np.asarray(Wo, dtype=np.float32).T.astype(bf)
    ident = np.eye(128, dtype=bf)
    bqa = np.asarray(bq, np.float32)
    bka = np.asarray(bk, np.float32)
    bva = np.asarray(bv, np.float32).astype(bf)

    def pack_w_hilo(wT_slice):
        # -> [128, E] uint16 holding (hi[1024] | lo[1024]) fp8 bytes
        hi, lo = _hilo(wT_slice)
        buf = np.empty((128, 2 * E), np.uint8)
        buf[:, 0:E] = _tile_w(hi).reshape(128, E).view(np.uint8)
        buf[:, E:2 * E] = _tile_w(lo).reshape(128, E).view(np.uint8)
        return buf.view(np.uint16)

    in_maps = []
    for c in range(N_CORES):
        s = slice(128 * c, 128 * (c + 1))
        wp1 = np.zeros((128, 2 * E + 4 + 128), dtype=bf)
        wp1u = wp1.view(np.uint16)
        wp1u[:, 0:E] = pack_w_hilo(WkT[:, s])
        wp1u[:, E:2 * E] = pack_w_hilo(WqT[:, s])
        wp1u[:, 2 * E:2 * E + 2] = bka[s].view(np.uint16).reshape(128, 2)
        wp1u[:, 2 * E + 2:2 * E + 4] = bqa[s].view(np.uint16).reshape(128, 2)
        wp1[0, 2 * E + 4:] = bva[s]
        wp2 = np.zeros((128, 2 * E + 128), dtype=bf)
        wp2.view(np.uint16)[:, 0:E] = pack_w_hilo(WvT[:, s])
        wp2[:, E:2 * E] = WoT[s, :]
        wp2[:, 2 * E:] = ident
        in_maps.append({
            "x1t": x1t, "x2t": x2t,
            "wp1": wp1, "wp2": wp2,
        })
    return in_maps


def kernel(x1, x2, Wq, bq, Wk, bk, Wv, bv, Wo, bo):
    nc = _get_nc()
    in_maps = make_in_maps(x1, x2, Wq, bq, Wk, bk, Wv, bv, Wo)
    res = run_bass_kernel_spmd(nc, in_maps, list(range(N_CORES)))
    ytf = res.results[0]["yt"].astype(np.float64)
    for c in range(1, N_CORES):
        ytf += res.results[c]["yt"].astype(np.float64)
    y = ytf.T.astype(np.float32) + np.asarray(bo, np.float32)[None, :]
    return y.reshape(B, SQ, E)


# revision 40
# speedup vs baseline: 1.0666x; 1.0625x over previous
"""MultiHeadCrossAttention on 8 TRN2 NeuronCores.

Sharding: tensor-parallel over heads (16 heads -> 2 per core); each core
writes a full-size partial of y.T which the host sums (replaces the
all-reduce). Design vs the f32r baseline (254us -> ~196us cost-model):

  * Activations bf16 end-to-end (tolerance is 2e-2; bf16 lands ~6e-3),
    halving DMA traffic vs f32.
  * q/k/v projections run fp8e4m3 DoubleRow (0.5 cyc/row, 2 contraction
    chunks per pass) with hi+lo splitting: x @ W ~ xhi@Whi + xlo@Whi +
    xhi@Wlo (lo*lo dropped). 12 DR matmuls replace 16 bf16-equivalents per
    512-col chunk (-25% PE) and land MORE accurate than bf16 (~1.3e-3).
    W is host-prescaled x32 so the fp8 lo plane clears the subnormal floor;
    the PSUM->SBUF bias step multiplies by 1/32. (Plain-fp8 scores/attnV/
    out-proj all FAIL the 2e-2 budget -- measured 3-6e-2 -- so everything
    else stays bf16.)
  * V projection swaps matmul roles (stationary = x2 chunk, moving = Wv) so
    V lands in PSUM already [kv, d]-oriented: no PE transposes for V.
  * attn@V reoriented: stationary = P.T [128kv, 128q] block, moving =
    [V|1] [128kv, 65] -> out [128q, 65]: 66.5k PE cycles instead of 131k,
    and the softmax denominator rides along as the ones column.
  * Normalize is per-partition (q on partitions): vector reciprocal of the
    denominator column + tensor_scalar_mul; no gpsimd broadcast.
  * attn tiles are PE-transposed back to [d, q] for the out-projection.
  * Weights/biases packed into two DMA transfers (wp1/wp2); f32 bias bytes
    and fp8 weight planes live inside the bf16 pack via bitcast views.

Schedule: exp on ACT is the pacing stream (~139us busy; ACT is the only
engine with exp, 0.83ns/row + 185ns/inst PSUM/SBUF access). Windows of 16
scores-matmul+exp steps are interleaved with the previous window's attn@V
steps, and a filler queue drips projection / out-proj / DMA work into each
step so PE (~151us busy) stays dense. PSUM budget (8 banks): scores 2x2,
attn@V out 2x1, everything else shares a 2x1 ring.
"""
import numpy as np
import ml_dtypes
from collections import deque
from contextlib import ExitStack

import concourse.bass as bass
import concourse.mybir as mybir
import concourse.tile as tile
from concourse import bacc
from concourse.bass_utils import run_bass_kernel_spmd

N_CORES = 8
B, SQ, SKV, E, DH = 4, 1024, 2048, 1024, 64
Q_ROWS = B * SQ      # 4096
KV_ROWS = B * SKV    # 8192
EC = E // 128        # 8 contraction chunks
QC = Q_ROWS // 512   # 8 q column chunks
KVC_B = SKV // 128   # 16 kv blocks per batch
F32 = mybir.dt.float32
BF16 = mybir.dt.bfloat16
FP8 = mybir.dt.float8e4
DR = mybir.MatmulPerfMode.DoubleRow
Exp = mybir.ActivationFunctionType.Exp
SHIFT = 0.0

_CACHE = {}


def _build():
    nc = bacc.Bacc("TRN2", target_bir_lowering=False, debug=False,
                   num_devices=N_CORES)
    # x slabs as fp8 hi/lo pairs (same bytes as bf16, but projections can run
    # DoubleRow: 2 contraction chunks per pass at 0.5 cyc/row)
    x1t = nc.dram_tensor("x1t", [QC, 128, 2, EC, 512], FP8,
                         kind="ExternalInput").ap()
    x2t = nc.dram_tensor("x2t", [KV_ROWS // 512, 128, 2, EC, 512], FP8,
                         kind="ExternalInput").ap()
    # packed weights: one DMA dispatch each instead of ~10 small ones
    # wp1 = [Wk hi|lo fp8 | Wq hi|lo fp8 | bk | bq | bv-row(row0)]
    # wp2 = [Wv hi|lo fp8 | Wo.T bf16 | identity bf16]
    wp1 = nc.dram_tensor("wp1", [128, E + E + 4 + 128], BF16,
                         kind="ExternalInput").ap()
    wp2 = nc.dram_tensor("wp2", [128, E + E + 128], BF16,
                         kind="ExternalInput").ap()
    yt = nc.dram_tensor("yt", [E, Q_ROWS], BF16, kind="ExternalOutput").ap()
    yt_r = yt.rearrange("(oc p) q -> p oc q", p=128)

    with tile.TileContext(nc) as tc, ExitStack() as ctx:
        const = ctx.enter_context(tc.tile_pool(name="const", bufs=1))
        persist = ctx.enter_context(tc.tile_pool(name="persist", bufs=1))
        ptp = ctx.enter_context(tc.tile_pool(name="ptp", bufs=2))
        xload = ctx.enter_context(tc.tile_pool(name="xload", bufs=6))
        work = ctx.enter_context(tc.tile_pool(name="work", bufs=3))
        ps_pj = ctx.enter_context(tc.tile_pool(name="ps_pj", bufs=2, space="PSUM"))
        ps_s = ctx.enter_context(tc.tile_pool(name="ps_s", bufs=2, space="PSUM"))
        ps_o = ctx.enter_context(tc.tile_pool(name="ps_o", bufs=2, space="PSUM"))

        wp1_sb = const.tile([128, E + E + 4 + 128], BF16, tag="wp1")
        wp2_sb = const.tile([128, E + E + 128], BF16, tag="wp2")
        bv_row = const.tile([128, 128], BF16, tag="bvrow")
        # first packed-weight load goes through the ACT DGE queue so the SP
        # queue starts on the big x-slab loads immediately
        nc.scalar.dma_start(wp1_sb[:], wp1[:])
        # fp8 hi/lo weight planes live in the bf16-typed pack; bitcast views.
        # Weight values are pre-scaled x32 on host (fp8 subnormal floor); the
        # bias step multiplies PSUM by 1/32.
        wk_sb = wp1_sb[:, 0:E].bitcast(FP8).rearrange(
            "p (hl ec c) -> p hl ec c", hl=2, c=128)
        wq_sb = wp1_sb[:, E:2 * E].bitcast(FP8).rearrange(
            "p (hl ec c) -> p hl ec c", hl=2, c=128)
        # f32 bias bytes live in two bf16 slots each; reinterpret in place
        bk_sb = wp1_sb[:, 2 * E:2 * E + 2].bitcast(F32)
        bq_sb = wp1_sb[:, 2 * E + 2:2 * E + 4].bitcast(F32)
        bvr_sb = wp1_sb[0:1, 2 * E + 4:2 * E + 4 + 128]
        wv_sb = wp2_sb[:, 0:E].bitcast(FP8).rearrange(
            "p (hl ec c) -> p hl ec c", hl=2, c=128)
        wo_sb = wp2_sb[:, E:2 * E]
        id_sb = wp2_sb[:, 2 * E:2 * E + 128]
        nc.gpsimd.partition_broadcast(bv_row[:], bvr_sb[:])

        qt_sb = persist.tile([128, QC, 512], BF16, tag="qt", name="qt")
        kt_sb = [persist.tile([128, SKV], BF16, tag=f"kt{b}", name=f"kt{b}")
                 for b in range(B)]
        v_sb = [persist.tile([128, KVC_B, 130], BF16, tag=f"v{b}",
                             name=f"v{b}") for b in range(B)]
        at_sb = [persist.tile([128, 8, 128], BF16, tag=f"at{b}",
                              name=f"at{b}") for b in range(B)]
        att_T = [persist.tile([128, SQ], BF16, tag=f"aT{b}", name=f"aT{b}")
                 for b in range(B)]
        # softmax-denominator ones columns (cols 64 and 129 of each kv block)
        for b in range(B):
            nc.gpsimd.memset(v_sb[b][:, :, 64::65], 1.0)

        xq = {}     # qc -> xload tile
        xkv = {}    # (b, j) -> xload tile
        qps = {}
        kps = {}
        vps = {}

        fillers = deque()

        def drain(n):
            for _ in range(min(n, len(fillers))):
                fillers.popleft()()

        def load_x1(qc):
            xt = xload.tile([128, 2, EC, 512], FP8, tag="x", name=f"xq{qc}")
            nc.sync.dma_start(xt[:], x1t[qc])
            xq[qc] = xt

        def load_x2(b, j):
            xt = xload.tile([128, 2, EC, 512], FP8, tag="x",
                            name=f"xkv{b}_{j}")
            nc.sync.dma_start(xt[:], x2t[b * 4 + j])
            xkv[(b, j)] = xt

        # hi/lo fp8 DoubleRow projection: x@W ~ xhi@Whi + xlo@Whi + xhi@Wlo
        # (lo*lo dropped), each DR matmul covers 2 contraction chunks.
        HL = ((0, 0), (1, 0), (0, 1))   # (x plane, w plane)

        def proj_dr(psum, w4, xt, cols, cps, last):
            for i, cp in enumerate(cps):
                for k, (xhl, whl) in enumerate(HL):
                    nc.tensor.matmul(
                        psum, w4[:, whl, cp:cp + 2, :],
                        xt[:, xhl, cp:cp + 2, cols],
                        start=(cp == 0 and k == 0),
                        stop=(last and i == len(cps) - 1 and k == len(HL) - 1),
                        perf_mode=DR)

        def proj_q_mm(qc, half):
            if half == 0:
                qps[qc] = ps_pj.tile([128, 512], F32, tag="pj", name=f"qps{qc}")
            proj_dr(qps[qc][:], wq_sb, xq[qc], slice(0, 512),
                    (0, 2) if half == 0 else (4, 6), half == 1)

        def proj_q_bias(qc):
            nc.vector.tensor_scalar(qt_sb[:, qc, :], qps[qc][:], 1.0 / 32,
                                    bq_sb[:], mybir.AluOpType.mult,
                                    mybir.AluOpType.add)

        def proj_k_mm(b, j, half):
            if half == 0:
                kps[(b, j)] = ps_pj.tile([128, 512], F32, tag="pj",
                                         name=f"kps{b}_{j}")
            proj_dr(kps[(b, j)][:], wk_sb, xkv[(b, j)], slice(0, 512),
                    (0, 2) if half == 0 else (4, 6), half == 1)

        def proj_k_bias(b, j):
            nc.vector.tensor_scalar(kt_sb[b][:, j * 512:(j + 1) * 512],
                                    kps[(b, j)][:], 1.0 / 32, bk_sb[:],
                                    mybir.AluOpType.mult, mybir.AluOpType.add)

        def proj_v_blk(b, j, t):
            # swapped-role projection: stationary = x2 chunk, moving = Wv
            # -> V comes out of PSUM already [kv, d]; no transpose needed
            kc = j * 4 + t
            vp = ps_pj.tile([128, 128], F32, tag="pj", name=f"vps{b}_{kc}")
            cols = slice(t * 128, (t + 1) * 128)
            for cp in (0, 2, 4, 6):
                for k, (xhl, whl) in enumerate(HL):
                    nc.tensor.matmul(
                        vp[:], xkv[(b, j)][:, xhl, cp:cp + 2, cols],
                        wv_sb[:, whl, cp:cp + 2, :],
                        start=(cp == 0 and k == 0),
                        stop=(cp == 6 and k == len(HL) - 1),
                        perf_mode=DR)
            dst = v_sb[b][:, kc].rearrange("p (h x) -> p h x", h=2)
            r2 = "p (h x) -> p h x"
            nc.vector.scalar_tensor_tensor(
                dst[:, :, 0:64], vp[:].rearrange(r2, h=2), 1.0 / 32,
                bv_row[:].rearrange(r2, h=2),
                mybir.AluOpType.mult, mybir.AluOpType.add)

        def oproj_o(b, g, o):
            yp = ps_pj.tile([128, 512], F32, tag="pj", name=f"yps{b}_{g}_{o}")
            nc.tensor.matmul(yp[:], wo_sb[:, o * 128:(o + 1) * 128],
                             att_T[b][:, g * 512:(g + 1) * 512],
                             start=True, stop=True)
            ysb = work.tile([128, 512], BF16, tag="y", bufs=9,
                            name=f"ysb{b}_{g}_{o}")
            if b == B - 1 and o % 2 == 0:
                # tail: ACT is idle after the last exp; alternate with DVE
                nc.scalar.copy(ysb[:], yp[:])
            else:
                nc.vector.tensor_copy(ysb[:], yp[:])
            nc.sync.dma_start(
                yt_r[:, o, b * SQ + g * 512: b * SQ + (g + 1) * 512], ysb[:])

        def push_qproj(qc, load=True):
            out = []
            if load:
                out.append(lambda: load_x1(qc))
            out.append(lambda: proj_q_mm(qc, 0))
            out.append(lambda: (proj_q_mm(qc, 1), proj_q_bias(qc)))
            return out

        def push_kproj(b, js=range(4), load=True):
            out = []
            for j in js:
                if load:
                    out.append(lambda b=b, j=j: load_x2(b, j))
                out.append(lambda b=b, j=j: proj_k_mm(b, j, 0))
                out.append(lambda b=b, j=j: (proj_k_mm(b, j, 1),
                                             proj_k_bias(b, j)))
            return out

        def push_vproj(b):
            return [lambda b=b, j=j, t=t: proj_v_blk(b, j, t)
                    for j in range(4) for t in range(4)]

        def push_oproj(b, gs=(0, 1)):
            return [lambda b=b, g=g, o=o: oproj_o(b, g, o)
                    for g in gs for o in range(EC)]

        def interleave(*lists):
            # round-robin so slow-consumer thunks (oproj) never cluster on
            # the 2-deep pj PSUM ring
            lists = [list(x) for x in lists if x]
            while lists:
                for x in list(lists):
                    fillers.append(x.pop(0))
                    if not x:
                        lists.remove(x)

        pts = {}

        def scores_steps(b, h, u_split=False):
            pt = ptp.tile([128, KVC_B, SQ], BF16, tag="pt", name=f"pt{b}_{h}")
            pts[(b, h)] = pt
            if u_split:
                # startup window: per-u halves grouped by x2-slab arrival so
                # exp tracks the DMA landings as closely as possible
                for j in range(4):
                    for u in range(2):
                        for kc in range(4 * j, 4 * j + 4):
                            sp = ps_s.tile([128, 512], F32, tag="s",
                                           name=f"sps{b}_{h}_{kc}_{u}")
                            nc.tensor.matmul(
                                sp[:],
                                kt_sb[b][h * 64:h * 64 + 64,
                                         kc * 128:(kc + 1) * 128],
                                qt_sb[h * 64:h * 64 + 64, 2 * b + u, :],
                                start=True, stop=True)
                            nc.scalar.activation(
                                pt[:, kc, u * 512:(u + 1) * 512], sp[:], Exp,
                                bias=-SHIFT, scale=0.125)
                            yield
            else:
                for kc in range(KVC_B):
                    sp = ps_s.tile([128, SQ], F32, tag="s",
                                   name=f"sps{b}_{h}_{kc}")
                    for u in range(2):
                        nc.tensor.matmul(
                            sp[:, u * 512:(u + 1) * 512],
                            kt_sb[b][h * 64:h * 64 + 64,
                                     kc * 128:(kc + 1) * 128],
                            qt_sb[h * 64:h * 64 + 64, 2 * b + u, :],
                            start=True, stop=True)
                    nc.scalar.activation(pt[:, kc, :], sp[:], Exp,
                                         bias=-SHIFT, scale=0.125)
                    yield

        def attnv_steps(b, h):
            pt = pts[(b, h)]
            for qb in range(8):
                op = ps_o.tile([128, 65], F32, tag="o", name=f"o{b}_{h}_{qb}")
                for kc2 in range(KVC_B):
                    nc.tensor.matmul(
                        op[:], pt[:, kc2, qb * 128:(qb + 1) * 128],
                        v_sb[b][:, kc2, h * 65:h * 65 + 65],
                        start=(kc2 == 0), stop=(kc2 == KVC_B - 1))
                rc = work.tile([128, 1], F32, tag="rc", bufs=6,
                               name=f"rc{b}_{h}_{qb}")
                nc.vector.reciprocal(rc[:], op[:, 64:65])
                nc.vector.tensor_scalar_mul(at_sb[b][:, qb, h * 64:h * 64 + 64],
                                            op[:, 0:64], rc[:])
                if h == 1:
                    tp = ps_pj.tile([128, 128], BF16, tag="pj",
                                    name=f"tp{b}_{qb}")
                    nc.tensor.transpose(tp[:], at_sb[b][:, qb, :], id_sb[:])
                    nc.vector.tensor_copy(att_T[b][:, qb * 128:(qb + 1) * 128],
                                          tp[:])
                    if b == B - 1 and 3 <= qb < 7:
                        # spread g0 out-proj units over qb 3-6
                        for o in (2 * (qb - 3), 2 * (qb - 3) + 1):
                            oproj_o(b, 0, o)
                    elif b == B - 1 and qb == 7:
                        for o in range(EC):
                            oproj_o(b, 1, o)
                yield

        def drive(s, a_old, n_old, a_new, ds=2):
            # interleave the current window's scores/exp stream with the
            # previous window's attn@V stream.  The last TWO attnV steps are
            # carried past the window boundary and flushed one-per-step right
            # after the next window's first scores steps, so the boundary exp
            # never queues behind them.  The lag is FIXED at two steps
            # (consume 6 new + flush 2 old = produce 8 per window), so
            # nothing older than the immediately-previous window is ever
            # pending when a window's scores start writing the pt ring.
            k = 0
            acount = 0
            while s is not None:
                try:
                    next(s)
                    k += 1
                    drain(ds)
                except StopIteration:
                    s = None
                if n_old > 0:
                    try:
                        next(a_old)
                        drain(1)
                    except StopIteration:
                        pass
                    n_old -= 1
                if a_new is not None and k % 2 == 0 and acount < 6:
                    try:
                        next(a_new)
                        acount += 1
                        drain(1)
                    except StopIteration:
                        a_new = None
            return a_new, (8 - acount if a_new is not None else 0)

        # ---- startup: minimal critical path to the first exp ----
        load_x1(0)
        load_x2(0, 0)
        proj_k_mm(0, 0, 0)
        proj_k_mm(0, 0, 1)
        proj_k_bias(0, 0)
        proj_q_mm(0, 0)
        proj_q_mm(0, 1)
        proj_q_bias(0)
        load_x1(1)
        load_x2(0, 1)
        nc.scalar.dma_start(wp2_sb[:], wp2[:])
        load_x2(0, 2)
        load_x2(0, 3)
        proj_q_mm(1, 0)
        proj_q_mm(1, 1)
        proj_q_bias(1)
        # queue for batch-0/1 windows: remaining k(0), v(0), q(2,3), then
        # kv(1), kv(2), k(3) in emission-safe order (v(b) before any later
        # load that recycles b's xload slots)
        for t in push_kproj(0, js=range(1, 4), load=False):
            fillers.append(t)
        interleave(push_vproj(0), push_qproj(2) + push_qproj(3))
        for t in (push_kproj(1) + push_vproj(1) + push_kproj(2)):
            fillers.append(t)

        # Filler pushes are scheduled per window.  oproj(b) may only be
        # pushed once attnv(b,1) has been fully EMITTED (it reads att_T[b]),
        # which happens during the drive of the following window.
        windows = [(b, h) for b in range(B) for h in (0, 1)]
        pushes = {
            (1, 0): lambda: interleave(
                push_oproj(0),
                push_qproj(4) + push_qproj(5) + push_vproj(2)),
            (1, 1): lambda: interleave(push_kproj(3)),
            (2, 0): lambda: interleave(
                push_oproj(1),
                push_qproj(6) + push_qproj(7) + push_vproj(3)),
            (3, 0): lambda: interleave(push_oproj(2)),
        }
        old_a, old_n = None, 0   # carried remainder of attnv(i-2)
        new_a = None             # attnv(i-1), fresh each window
        for i, (b, h) in enumerate(windows):
            s = scores_steps(b, h, u_split=False)
            old_a, old_n = drive(s, old_a, old_n, new_a)
            new_a = attnv_steps(b, h)
            if (b, h) in pushes:
                pushes[(b, h)]()
        for g in (old_a, new_a):
            while g is not None:
                try:
                    next(g)
                    drain(1)
                except StopIteration:
                    g = None
        while fillers:
            drain(len(fillers))

    nc.compile()
    return nc


def _get_nc():
    if "nc" not in _CACHE:
        _CACHE["nc"] = _build()
    return _CACHE["nc"]


def _tile_x(xt2d, nchunks):
    # [E, R] -> [R/512, 128, EC, 512]
    return np.ascontiguousarray(
        xt2d.reshape(EC, 128, nchunks, 512).transpose(2, 1, 0, 3))


def _tile_w(wt_slice):
    # [E, 128] -> [128, EC, 128]
    return np.ascontiguousarray(
        wt_slice.reshape(EC, 128, 128).transpose(1, 0, 2))


def _hilo(a):
    f8 = ml_dtypes.float8_e4m3
    hi = a.astype(f8)
    lo = (a - hi.astype(np.float32)).astype(f8)
    return hi, lo


def _tile_x_hilo(xt2d, nchunks):
    # [E, R] f32 -> [R/512, 128, 2, EC, 512] fp8 (hi, lo planes)
    hi, lo = _hilo(xt2d)
    return np.ascontiguousarray(
        np.stack([_tile_x(hi, nchunks), _tile_x(lo, nchunks)], axis=2))


def make_in_maps(x1, x2, Wq, bq, Wk, bk, Wv, bv, Wo, bo=None):
    bf = ml_dtypes.bfloat16
    x1f = np.ascontiguousarray(np.asarray(x1, np.float32).reshape(Q_ROWS, E).T)
    x2f = np.ascontiguousarray(np.asarray(x2, np.float32).reshape(KV_ROWS, E).T)
    x1t = _tile_x_hilo(x1f, QC)
    x2t = _tile_x_hilo(x2f, KV_ROWS // 512)
    # weights scaled x32 so fp8 lo-planes stay above the subnormal floor
    WqT = np.asarray(Wq, dtype=np.float32).T * 32.0
    WkT = np.asarray(Wk, dtype=np.float32).T * 32.0
    WvT = np.asarray(Wv, dtype=np.float32).T * 32.0
    WoT = np.asarray(Wo, dtype=np.float32).T.astype(bf)
    ident = np.eye(128, dtype=bf)
    bqa = np.asarray(bq, np.float32)
    bka = np.asarray(bk, np.float32)
    bva = np.asarray(bv, np.float32).astype(bf)

    def pack_w_hilo(wT_slice):
        # -> [128, E] uint16 holding (hi[1024] | lo[1024]) fp8 bytes
        hi, lo = _hilo(wT_slice)
        buf = np.empty((128, 2 * E), np.uint8)
        buf[:, 0:E] = _tile_w(hi).reshape(128, E).view(np.uint8)
        buf[:, E:2 * E] = _tile_w(lo).reshape(128, E).view(np.uint8)
        return buf.view(np.uint16)

    in_maps = []
    for c in range(N_CORES):
        s = slice(128 * c, 128 * (c + 1))
        wp1 = np.zeros((128, 2 * E + 4 + 128), dtype=bf)
        wp1u = wp1.view(np.uint16)
        wp1u[:, 0:E] = pack_w_hilo(WkT[:, s])
        wp1u[:, E:2 * E] = pack_w_hilo(WqT[:, s])
        wp1u[:, 2 * E:2 * E + 2] = bka[s].view(np.uint16).reshape(128, 2)
        wp1u[:, 2 * E + 2:2 * E + 4] = bqa[s].view(np.uint16).reshape(128, 2)
        wp1[0, 2 * E + 4:] = bva[s]
        wp2 = np.zeros((128, 2 * E + 128), dtype=bf)
        wp2.view(np.uint16)[:, 0:E] = pack_w_hilo(WvT[:, s])
        wp2[:, E:2 * E] = WoT[s, :]
        wp2[:, 2 * E:] = ident
        in_maps.append({
            "x1t": x1t, "x2t": x2t,
            "wp1": wp1, "wp2": wp2,
        })
    return in_maps


def kernel(x1, x2, Wq, bq, Wk, bk, Wv, bv, Wo, bo):
    nc = _get_nc()
    in_maps = make_in_maps(x1, x2, Wq, bq, Wk, bk, Wv, bv, Wo)
    res = run_bass_kernel_spmd(nc, in_maps, list(range(N_CORES)))
    ytf = res.results[0]["yt"].astype(np.float64)
    for c in range(1, N_CORES):
        ytf += res.results[c]["yt"].astype(np.float64)
    y = ytf.T.astype(np.float32) + np.asarray(bo, np.float32)[None, :]
    return y.reshape(B, SQ, E)

